# revision 11
# baseline (speedup 1.0000x reference)
"""CRF (forward log-likelihood + Viterbi decode) Bass kernel for Trainium2.

Problem: B=256, T=512, K=128.
  reference(logits[B,T,K], tags[B,T], lengths[B], trans[K,K])
    -> (loss scalar, pred_sequence [B,T] int32, viterbi_score [B] f32)

Sharding: data-parallel over batch, 8 cores x 32 rows each; trans replicated.

Per-core device algorithm (validated against reference in numpy):
  Forward (exp domain, PE matmuls):
     u0=exp(logits[:,0]); z0=sum; p=u0/z0
     t>=1: q = p @ exp(trans); u = q*exp(logits[:,t]); z_t=sum_j u; p=u/z_t
     log_norm[b] = sum_{t<len_b} log z_t  (no masking needed inside the scan)
  Gold score via one-hot tricks with mask pre-folded into the tag values
     (masked tag := -1, which never matches the iota).
  Viterbi (exact f32 on vector engines):
     state v natural [32b, 128i]; per step: replicate v across partition
     groups with a tiny PE matmul, add pre-replicated trans, segmented
     max-reduce, un-permute with 4 selection matmuls, add logits, freeze by
     mask; store v_t to HBM scratch.
  Backtrack (exact): tag=argmax(v_last); per step one-hot matmul gathers
     trans[:, tag], add stored v_{t-1}, max8/max_index (first-max, matches
     jnp.argmax).
"""

import os

import numpy as np

B, T, K = 256, 512, 128
NCORES = 8
BL = B // NCORES  # 32 batch rows per core
G = K // BL       # 4 partition groups
CH = 32           # time-chunk for logits streaming

_BUILD_CACHE = {}


def build_bass(t_steps=T):
    """Build the per-core Bass program (SPMD: same NEFF, per-core shards)."""
    import concourse.bass as bass
    import concourse.bacc as bacc
    import concourse.mybir as mybir
    import concourse.tile as tile
    from concourse.masks import make_identity

    fp32 = mybir.dt.float32
    i32 = mybir.dt.int32
    u32 = mybir.dt.uint32
    AF = mybir.ActivationFunctionType
    ALU = mybir.AluOpType
    AX = mybir.AxisListType

    Tn = t_steps
    chs = min(CH, Tn)
    assert Tn % chs == 0
    nch = Tn // chs
    n_thi = 4
    tlo = chs // n_thi  # 8

    nc = bacc.Bacc("TRN2", target_bir_lowering=False)

    logits = nc.dram_tensor("logits", [BL, Tn, K], fp32, kind="ExternalInput")
    tags = nc.dram_tensor("tags", [BL, Tn], i32, kind="ExternalInput")
    lengths = nc.dram_tensor("lengths", [BL, 1], i32, kind="ExternalInput")
    trans = nc.dram_tensor("trans", [K, K], fp32, kind="ExternalInput")

    loss_o = nc.dram_tensor("loss_o", [1, 1], fp32, kind="ExternalOutput")
    path_o = nc.dram_tensor("path_o", [BL, Tn], i32, kind="ExternalOutput")
    vscore_o = nc.dram_tensor("vscore_o", [BL, 1], fp32, kind="ExternalOutput")

    vitbuf = nc.dram_tensor("vitbuf", [Tn, BL, K], fp32)  # scratch HBM

    with tile.TileContext(nc) as tc:
        with (
            tc.tile_pool(name="singles", bufs=1) as singles,
            tc.tile_pool(name="chunkraw", bufs=2) as chunkraw_p,
            tc.tile_pool(name="chunk32", bufs=2) as chunk32_p,
            tc.tile_pool(name="wide", bufs=2) as wide_p,
            tc.tile_pool(name="small", bufs=3) as small_p,
            tc.tile_pool(name="bt", bufs=3) as bt_p,
            tc.tile_pool(name="vload", bufs=8) as vload_p,
            tc.tile_pool(name="ps", bufs=2, space="PSUM") as ps,
        ):
            # ---------------- constants / setup ----------------
            id128 = singles.tile([128, 128], fp32)
            make_identity(nc, id128)
            id32 = singles.tile([32, 32], fp32)
            make_identity(nc, id32)
            ones128 = singles.tile([128, 1], fp32)
            nc.gpsimd.memset(ones128, 1.0)
            ones32 = singles.tile([32, 1], fp32)
            nc.gpsimd.memset(ones32, 1.0)

            # fold4[p, b'] = 1 if p % 32 == b'  ; repid = its transpose
            fold4 = singles.tile([128, 32], fp32)
            nc.vector.tensor_tensor(
                out=fold4, in0=id128[:, 0:32], in1=id128[:, 32:64], op=ALU.add)
            nc.vector.tensor_tensor(
                out=fold4, in0=fold4, in1=id128[:, 64:96], op=ALU.add)
            nc.vector.tensor_tensor(
                out=fold4, in0=fold4, in1=id128[:, 96:128], op=ALU.add)
            repid = singles.tile([32, 128], fp32)
            for g in range(G):
                nc.gpsimd.tensor_copy(out=repid[:, g * 32:(g + 1) * 32], in_=id32)

            # iotas (float compare domain everywhere)
            iotaT_i = singles.tile([BL, Tn], i32)
            nc.gpsimd.iota(iotaT_i, pattern=[[1, Tn]], base=0, channel_multiplier=0)
            iotaT_f = singles.tile([BL, Tn], fp32)
            nc.vector.tensor_copy(out=iotaT_f, in_=iotaT_i)
            iotaK_i = singles.tile([128, K], i32)
            nc.gpsimd.iota(iotaK_i, pattern=[[1, K]], base=0, channel_multiplier=0)
            iotaK_f = singles.tile([128, K], fp32)
            nc.vector.tensor_copy(out=iotaK_f, in_=iotaK_i)

            # trans in SBUF; E = exp(trans); transT
            transS = singles.tile([K, K], fp32)
            nc.sync.dma_start(out=transS, in_=trans[:])
            E = singles.tile([K, K], fp32)
            nc.scalar.activation(out=E, in_=transS, func=AF.Exp)
            transT_ps = ps.tile([K, K], fp32, tag="vr4P")
            nc.tensor.transpose(transT_ps, transS, id128)
            transT = singles.tile([K, K], fp32)
            nc.scalar.copy(out=transT, in_=transT_ps)

            # trans4[g*32+b, j_lo, i] = trans[i, g*32+j_lo] = transT[g*32+j_lo, i]
            # built by bouncing transT through HBM, then 4 broadcast loads
            transT_hbm = nc.dram_tensor("transT_hbm", [K, K], fp32)
            nc.sync.dma_start(out=transT_hbm[:], in_=transT)
            trans4 = singles.tile([128, 32, K], fp32)
            for g in range(G):
                src = bass.AP(
                    tensor=transT_hbm, offset=g * 32 * K,
                    ap=[[0, 32], [1, 32 * K]])
                nc.sync.dma_start(
                    out=trans4[g * 32:(g + 1) * 32].rearrange("p a b -> p (a b)"),
                    in_=src)

            # masks (f32)
            lenI = singles.tile([BL, 1], i32)
            nc.sync.dma_start(out=lenI, in_=lengths[:])
            lenF = singles.tile([BL, 1], fp32)
            nc.vector.tensor_copy(out=lenF, in_=lenI)
            mask32 = singles.tile([BL, Tn], fp32)
            nc.vector.tensor_tensor(
                out=mask32, in0=iotaT_f,
                in1=lenF[:].broadcast_to([BL, Tn]), op=ALU.is_lt)
            maskB32 = singles.tile([BL, Tn], fp32)
            nc.gpsimd.tensor_copy(out=maskB32, in_=mask32)
            nc.gpsimd.memset(maskB32[:, 0:1], 0.0)
            maskI = singles.tile([BL, Tn], mybir.dt.uint8)
            nc.vector.tensor_copy(out=maskI, in_=mask32)

            # tags: float; masked variants (masked slot -> -1, never matches iota)
            tagsI = singles.tile([BL, Tn], i32)
            nc.sync.dma_start(out=tagsI, in_=tags[:])
            tagsF = singles.tile([BL, Tn], fp32)
            nc.vector.tensor_copy(out=tagsF, in_=tagsI)
            tagmF = singles.tile([BL, Tn], fp32)  # (tags+1)*mask - 1
            nc.vector.scalar_tensor_tensor(
                out=tagmF, in0=tagsF, scalar=1.0, in1=mask32,
                op0=ALU.add, op1=ALU.mult)
            nc.vector.tensor_scalar_sub(tagmF, tagmF, 1.0)
            tagmPrevF = singles.tile([BL, Tn], fp32)  # masked (by maskB) prev tags
            prevF = singles.tile([BL, Tn], fp32)
            nc.gpsimd.memset(prevF[:, 0:1], 0.0)
            nc.gpsimd.tensor_copy(out=prevF[:, 1:Tn], in_=tagsF[:, 0:Tn - 1])
            nc.vector.scalar_tensor_tensor(
                out=tagmPrevF, in0=prevF, scalar=1.0, in1=maskB32,
                op0=ALU.add, op1=ALU.mult)
            nc.vector.tensor_scalar_sub(tagmPrevF, tagmPrevF, 1.0)

            # tags4m in [(t_hi,b), chunk, t_lo] layout for gold-unary
            tags4 = singles.tile([128, nch, tlo], fp32)
            for th in range(n_thi):
                src_t = bass.AP(
                    tensor=tagmF.tensor, offset=tagmF.offset + th * tlo,
                    ap=[tagmF.ap[0], [chs, nch], [1, tlo]])
                nc.sync.dma_start(out=tags4[th * 32:(th + 1) * 32], in_=src_t)

            # persistent state
            zbuf = singles.tile([BL, Tn], fp32)
            v_state = singles.tile([BL, K], fp32)
            uacc = singles.tile([128, 1], fp32)
            nc.gpsimd.memset(uacc, 0.0)
            pathF = singles.tile([BL, Tn], fp32)

            # ---------------- chunk load + gold-unary ----------------
            def load_chunk(c):
                raw128 = chunkraw_p.tile([128, tlo, K], fp32, tag="raw128")
                for th in range(n_thi):
                    nc.sync.dma_start(
                        out=raw128[th * 32:(th + 1) * 32],
                        in_=logits[:, c * chs + th * tlo: c * chs + (th + 1) * tlo, :])
                raw32 = chunk32_p.tile([BL, chs, K], fp32, tag="raw32")
                nc.sync.dma_start(out=raw32, in_=logits[:, c * chs:(c + 1) * chs, :])
                exp32 = chunk32_p.tile([BL, chs, K], fp32, tag="exp32")
                nc.scalar.activation(out=exp32, in_=raw32, func=AF.Exp)
                return raw128, raw32, exp32

            def gold_unary(c, raw128):
                oh = wide_p.tile([128, tlo, K], fp32, tag="goldoh")
                nc.vector.tensor_tensor(
                    out=oh,
                    in0=iotaK_f[:].unsqueeze(1).broadcast_to([128, tlo, K]),
                    in1=tags4[:, c, :].unsqueeze(2).broadcast_to([128, tlo, K]),
                    op=ALU.is_equal)
                selu = wide_p.tile([128, tlo, K], fp32, tag="selu")
                usr = small_p.tile([128, 1], fp32, tag="usr")
                nc.vector.scalar_tensor_tensor(
                    out=selu, in0=oh, scalar=1.0, in1=raw128,
                    op0=ALU.mult, op1=ALU.mult, accum_out=usr)
                nc.vector.tensor_tensor(out=uacc, in0=uacc, in1=usr, op=ALU.add)

            raw128, raw32, exp32 = load_chunk(0)
            gold_unary(0, raw128)

            # forward init (t=0)
            nc.vector.tensor_reduce(
                out=zbuf[:, 0:1], in_=exp32[:, 0, :], axis=AX.X, op=ALU.add)
            r0 = small_p.tile([BL, 1], fp32, tag="recip")
            nc.vector.reciprocal(out=r0, in_=zbuf[:, 0:1])
            p0 = small_p.tile([BL, K], fp32, tag="pstate")
            nc.scalar.activation(out=p0, in_=exp32[:, 0, :], func=AF.Copy, scale=r0)
            pT_ps = ps.tile([K, BL], fp32, tag="pT")
            nc.tensor.transpose(pT_ps, p0, id32)
            pTs = small_p.tile([K, BL], fp32, tag="pTs")
            nc.scalar.copy(out=pTs, in_=pT_ps)

            # viterbi init
            nc.vector.tensor_copy(out=v_state, in_=raw32[:, 0, :])
            nc.sync.dma_start(out=vitbuf[0], in_=v_state)

            # ---------------- main scan ----------------
            JR = 12  # j_lo split for the M1 reduce: DVE [0:JR), Pool [JR:32)
            for t in range(1, Tn):
                c, tl = divmod(t, chs)
                if tl == 0:
                    raw128, raw32, exp32 = load_chunk(c)
                    gold_unary(c, raw128)

                # ---- forward (lse in exp domain) ----
                qP = ps.tile([BL, K], fp32, tag="qP")
                nc.tensor.matmul(qP, pTs, E, start=True, stop=True)
                u = small_p.tile([BL, K], fp32, tag="ufwd")
                nc.vector.scalar_tensor_tensor(
                    out=u, in0=qP, scalar=1.0, in1=exp32[:, tl, :],
                    op0=ALU.mult, op1=ALU.mult, accum_out=zbuf[:, t:t + 1])
                r = small_p.tile([BL, 1], fp32, tag="recip")
                nc.vector.reciprocal(out=r, in_=zbuf[:, t:t + 1])
                p = small_p.tile([BL, K], fp32, tag="pstate")
                nc.scalar.activation(out=p, in_=u, func=AF.Copy, scale=r)
                pT_ps = ps.tile([K, BL], fp32, tag="pT")
                nc.tensor.transpose(pT_ps, p, id32)
                pTs = small_p.tile([K, BL], fp32, tag="pTs")
                nc.scalar.copy(out=pTs, in_=pT_ps)

                # ---- viterbi step ----
                vr4P = ps.tile([128, K], fp32, tag="vr4P")
                nc.tensor.matmul(vr4P, repid, v_state, start=True, stop=True)
                vr4 = small_p.tile([128, K], fp32, tag="vr4s")
                nc.scalar.copy(out=vr4, in_=vr4P)
                S = wide_p.tile([128, 32, K], fp32, tag="S")
                nc.vector.tensor_tensor(
                    out=S, in0=vr4[:].unsqueeze(1).broadcast_to([128, 32, K]),
                    in1=trans4, op=ALU.add)
                M = small_p.tile([128, 32], fp32, tag="M")
                nc.vector.tensor_reduce(out=M, in_=S, axis=AX.X, op=ALU.max)
                vnP = ps.tile([BL, K], fp32, tag="vnP")
                for g in range(G):
                    nc.tensor.matmul(
                        vnP[:, g * 32:(g + 1) * 32],
                        id128[:, g * 32:(g + 1) * 32], M,
                        start=True, stop=True)
                u2 = small_p.tile([BL, K], fp32, tag="u2")
                nc.vector.tensor_tensor(
                    out=u2, in0=vnP, in1=raw32[:, tl, :], op=ALU.add)
                nc.vector.copy_predicated(
                    out=v_state,
                    mask=maskI[:, t:t + 1].broadcast_to([BL, K]),
                    data=u2)
                nc.sync.dma_start(out=vitbuf[t], in_=v_state)

            # ---------------- forward finalize: log_norm ----------------
            logz = singles.tile([BL, Tn], fp32)
            nc.scalar.activation(out=logz, in_=zbuf, func=AF.Ln)
            nc.vector.tensor_tensor(out=logz, in0=logz, in1=mask32, op=ALU.mult)
            lognorm = singles.tile([BL, 1], fp32)
            nc.vector.tensor_reduce(out=lognorm, in_=logz, axis=AX.X, op=ALU.add)

            # ---------------- gold binary: one-hot pair-count matmuls ----------
            tch = min(128, Tn)
            ntch = Tn // tch
            tagsT = singles.tile([tch, ntch, BL], fp32)
            tagsPT = singles.tile([tch, ntch, BL], fp32)
            for cc in range(ntch):
                for (dst, srcb) in ((tagsT, tagsF), (tagsPT, tagmPrevF)):
                    tp = ps.tile([tch, BL], fp32, tag="pT")
                    nc.tensor.transpose(
                        tp, srcb[:, cc * tch:(cc + 1) * tch], id32)
                    nc.scalar.copy(out=dst[:, cc, :], in_=tp)
            bsc = singles.tile([128, BL], fp32)
            for b in range(BL):
                cP = ps.tile([K, K], fp32, tag="vr4P")
                for cc in range(ntch):
                    ohn = bt_p.tile([tch, K], fp32, tag="ohn")
                    nc.vector.tensor_tensor(
                        out=ohn, in0=iotaK_f[0:tch],
                        in1=tagsT[:, cc, b:b + 1].broadcast_to([tch, K]),
                        op=ALU.is_equal)
                    ohp = bt_p.tile([tch, K], fp32, tag="ohp")
                    nc.vector.tensor_tensor(
                        out=ohp, in0=iotaK_f[0:tch],
                        in1=tagsPT[:, cc, b:b + 1].broadcast_to([tch, K]),
                        op=ALU.is_equal)
                    nc.tensor.matmul(cP, ohp, ohn, start=(cc == 0),
                                     stop=(cc == ntch - 1))
                ct = bt_p.tile([K, K], fp32, tag="ct")
                nc.vector.tensor_tensor(out=ct, in0=cP, in1=transS, op=ALU.mult)
                nc.vector.tensor_reduce(
                    out=bsc[:, b:b + 1], in_=ct, axis=AX.X, op=ALU.add)

            # fold gold scores and loss
            useqP = ps.tile([BL, 1], fp32, tag="vnP")
            nc.tensor.matmul(useqP, fold4, uacc, start=True, stop=True)
            browP = ps.tile([1, BL], fp32, tag="qP")
            nc.tensor.matmul(browP, ones128, bsc, start=True, stop=True)
            brow = small_p.tile([1, BL], fp32, tag="brow")
            nc.scalar.copy(out=brow, in_=browP)
            bseqP = ps.tile([BL, 1], fp32, tag="pT")
            nc.tensor.transpose(bseqP, brow, ones32[0:1, 0:1])
            nllb = small_p.tile([BL, 1], fp32, tag="nllb")
            nc.vector.tensor_tensor(out=nllb, in0=lognorm, in1=useqP,
                                    op=ALU.subtract)
            nc.vector.tensor_tensor(out=nllb, in0=nllb, in1=bseqP,
                                    op=ALU.subtract)
            nllS = small_p.tile([BL, 1], fp32, tag="nllS")
            nc.vector.tensor_copy(out=nllS, in_=nllb)
            lossP = ps.tile([1, 1], fp32, tag="qP")
            nc.tensor.matmul(lossP, nllS, ones32, start=True, stop=True)
            lossS = small_p.tile([1, 1], fp32, tag="lossS")
            nc.scalar.copy(out=lossS, in_=lossP)
            nc.sync.dma_start(out=loss_o[:], in_=lossS)

            # ---------------- viterbi finalize + backtrack ----------------
            mx8 = small_p.tile([BL, 8], fp32, tag="mx8")
            nc.vector.max(out=mx8, in_=v_state)
            vs = small_p.tile([BL, 1], fp32, tag="vs")
            nc.vector.tensor_copy(out=vs, in_=mx8[:, 0:1])
            nc.sync.dma_start(out=vscore_o[:], in_=vs)
            idx8 = small_p.tile([BL, 8], u32, tag="idx8")
            nc.vector.max_index(out=idx8, in_max=mx8, in_values=v_state)
            nc.vector.tensor_copy(out=pathF[:, Tn - 1:Tn], in_=idx8[:, 0:1])

            for t in range(Tn - 1, 0, -1):
                vprev = vload_p.tile([BL, K], fp32, tag="vprev")
                nc.sync.dma_start(out=vprev, in_=vitbuf[t - 1])
                ohb = bt_p.tile([BL, K], fp32, tag="ohb")
                nc.vector.tensor_tensor(
                    out=ohb, in0=iotaK_f[0:BL],
                    in1=pathF[:, t:t + 1].broadcast_to([BL, K]),
                    op=ALU.is_equal)
                ohtP = ps.tile([K, BL], fp32, tag="pT")
                nc.tensor.transpose(ohtP, ohb, id32)
                ohs = bt_p.tile([K, BL], fp32, tag="ohs")
                nc.scalar.copy(out=ohs, in_=ohtP)
                gtP = ps.tile([BL, K], fp32, tag="qP")
                nc.tensor.matmul(gtP, ohs, transT, start=True, stop=True)
                w = bt_p.tile([BL, K], fp32, tag="w")
                nc.vector.tensor_tensor(out=w, in0=gtP, in1=vprev, op=ALU.add)
                wm8 = bt_p.tile([BL, 8], fp32, tag="wm8")
                nc.vector.max(out=wm8, in_=w)
                wi8 = bt_p.tile([BL, 8], u32, tag="wi8")
                nc.vector.max_index(out=wi8, in_max=wm8, in_values=w)
                wf = bt_p.tile([BL, 1], fp32, tag="wf")
                nc.vector.tensor_copy(out=wf, in_=wi8[:, 0:1])
                nc.vector.tensor_copy(out=pathF[:, t - 1:t], in_=pathF[:, t:t + 1])
                nc.vector.copy_predicated(
                    out=pathF[:, t - 1:t], mask=maskI[:, t:t + 1], data=wf)

            pathI = singles.tile([BL, Tn], i32)
            nc.vector.tensor_copy(out=pathI, in_=pathF)
            nc.sync.dma_start(out=path_o[:], in_=pathI)

    nc.finalize()
    return nc


def kernel(logits, tags, lengths, trans):
    from concourse.bass_utils import run_bass_kernel_spmd

    logits = np.ascontiguousarray(logits, dtype=np.float32)
    tags = np.ascontiguousarray(tags, dtype=np.int32)
    lengths = np.ascontiguousarray(lengths, dtype=np.int32)
    trans = np.ascontiguousarray(trans, dtype=np.float32)

    if "nc" not in _BUILD_CACHE:
        _BUILD_CACHE["nc"] = build_bass(T)
    nc = _BUILD_CACHE["nc"]

    in_maps = []
    for c in range(NCORES):
        sl = slice(c * BL, (c + 1) * BL)
        in_maps.append({
            "logits": logits[sl],
            "tags": tags[sl],
            "lengths": lengths[sl].reshape(BL, 1),
            "trans": trans,
        })
    res = run_bass_kernel_spmd(
        nc, in_maps, core_ids=list(range(NCORES)),
        trace=bool(int(os.environ.get("CRF_TRACE", "0"))))
    outs = res.results

    loss = np.float32(sum(float(o["loss_o"][0, 0]) for o in outs))
    path = np.concatenate([o["path_o"] for o in outs], axis=0)
    vscore = np.concatenate([o["vscore_o"][:, 0] for o in outs], axis=0)
    kernel.last_exec_time_ns = res.exec_time_ns
    return loss, path, vscore


kernel.last_exec_time_ns = None


# revision 13
# speedup vs baseline: 2.4332x; 2.4332x over previous
"""CRF (forward log-likelihood + Viterbi decode) Bass kernel for Trainium2.

Problem: B=256, T=512, K=128.
  reference(logits[B,T,K], tags[B,T], lengths[B], trans[K,K])
    -> (loss scalar, pred_sequence [B,T] int32, viterbi_score [B] f32)

Sharding: data-parallel over batch, 8 cores x 32 rows each; trans replicated.

Per-core device algorithm (validated against reference in numpy):
  Forward (exp domain, PE matmuls):
     u0=exp(logits[:,0]); z0=sum; p=u0/z0
     t>=1: q = p @ exp(trans); u = q*exp(logits[:,t]); z_t=sum_j u; p=u/z_t
     log_norm[b] = sum_{t<len_b} log z_t  (no masking needed inside the scan)
  Gold score via one-hot tricks with mask pre-folded into the tag values
     (masked tag := -1, which never matches the iota).
  Viterbi (exact f32 on vector engines):
     state v natural [32b, 128i]; per step: replicate v across partition
     groups with a tiny PE matmul, add pre-replicated trans, segmented
     max-reduce, un-permute with 4 selection matmuls, add logits, freeze by
     mask; store v_t to HBM scratch.
  Backtrack (exact): tag=argmax(v_last); per step one-hot matmul gathers
     trans[:, tag], add stored v_{t-1}, max8/max_index (first-max, matches
     jnp.argmax).
"""

import os

import numpy as np

B, T, K = 256, 512, 128
NCORES = 8
BL = B // NCORES  # 32 batch rows per core
G = K // BL       # 4 partition groups
CH = 32           # time-chunk for logits streaming

_BUILD_CACHE = {}


def build_bass(t_steps=T):
    """Build the per-core Bass program (SPMD: same NEFF, per-core shards)."""
    import concourse.bass as bass
    import concourse.bacc as bacc
    import concourse.mybir as mybir
    import concourse.tile as tile
    from concourse.masks import make_identity

    fp32 = mybir.dt.float32
    i32 = mybir.dt.int32
    u32 = mybir.dt.uint32
    AF = mybir.ActivationFunctionType
    ALU = mybir.AluOpType
    AX = mybir.AxisListType

    Tn = t_steps
    chs = min(CH, Tn)
    assert Tn % chs == 0
    nch = Tn // chs
    n_thi = 4
    tlo = chs // n_thi  # 8

    nc = bacc.Bacc("TRN2", target_bir_lowering=False)

    logits = nc.dram_tensor("logits", [BL, Tn, K], fp32, kind="ExternalInput")
    tags = nc.dram_tensor("tags", [BL, Tn], i32, kind="ExternalInput")
    lengths = nc.dram_tensor("lengths", [BL, 1], i32, kind="ExternalInput")
    trans = nc.dram_tensor("trans", [K, K], fp32, kind="ExternalInput")

    loss_o = nc.dram_tensor("loss_o", [1, 1], fp32, kind="ExternalOutput")
    path_o = nc.dram_tensor("path_o", [BL, Tn], i32, kind="ExternalOutput")
    vscore_o = nc.dram_tensor("vscore_o", [BL, 1], fp32, kind="ExternalOutput")

    vitbuf = nc.dram_tensor("vitbuf", [Tn, BL, K], fp32)  # scratch HBM

    with tile.TileContext(nc) as tc:
        with (
            tc.tile_pool(name="singles", bufs=1) as singles,
            tc.tile_pool(name="chunkraw", bufs=2) as chunkraw_p,
            tc.tile_pool(name="chunk32", bufs=2) as chunk32_p,
            tc.tile_pool(name="wide", bufs=2) as wide_p,
            tc.tile_pool(name="small", bufs=3) as small_p,
            tc.tile_pool(name="bt", bufs=3) as bt_p,
            tc.tile_pool(name="vload", bufs=8) as vload_p,
            tc.tile_pool(name="ps", bufs=2, space="PSUM") as ps,
        ):
            # ---------------- constants / setup ----------------
            id128 = singles.tile([128, 128], fp32)
            make_identity(nc, id128)
            id32 = singles.tile([32, 32], fp32)
            make_identity(nc, id32)
            ones128 = singles.tile([128, 1], fp32)
            nc.gpsimd.memset(ones128, 1.0)
            ones32 = singles.tile([32, 1], fp32)
            nc.gpsimd.memset(ones32, 1.0)

            # fold4[p, b'] = 1 if p % 32 == b'  ; repid = its transpose
            fold4 = singles.tile([128, 32], fp32)
            nc.vector.tensor_tensor(
                out=fold4, in0=id128[:, 0:32], in1=id128[:, 32:64], op=ALU.add)
            nc.vector.tensor_tensor(
                out=fold4, in0=fold4, in1=id128[:, 64:96], op=ALU.add)
            nc.vector.tensor_tensor(
                out=fold4, in0=fold4, in1=id128[:, 96:128], op=ALU.add)
            repid = singles.tile([32, 128], fp32)
            for g in range(G):
                nc.gpsimd.tensor_copy(out=repid[:, g * 32:(g + 1) * 32], in_=id32)

            # iotas (float compare domain everywhere)
            iotaT_i = singles.tile([BL, Tn], i32)
            nc.gpsimd.iota(iotaT_i, pattern=[[1, Tn]], base=0, channel_multiplier=0)
            iotaT_f = singles.tile([BL, Tn], fp32)
            nc.vector.tensor_copy(out=iotaT_f, in_=iotaT_i)
            iotaK_i = singles.tile([128, K], i32)
            nc.gpsimd.iota(iotaK_i, pattern=[[1, K]], base=0, channel_multiplier=0)
            iotaK_f = singles.tile([128, K], fp32)
            nc.vector.tensor_copy(out=iotaK_f, in_=iotaK_i)

            # trans in SBUF; E = exp(trans); transT
            transS = singles.tile([K, K], fp32)
            nc.sync.dma_start(out=transS, in_=trans[:])
            E = singles.tile([K, K], fp32)
            nc.scalar.activation(out=E, in_=transS, func=AF.Exp)
            transT_ps = ps.tile([K, K], fp32, tag="vr4P")
            nc.tensor.transpose(transT_ps, transS, id128)
            transT = singles.tile([K, K], fp32)
            nc.scalar.copy(out=transT, in_=transT_ps)

            # trans4[g*32+b, j_lo, i] = trans[i, g*32+j_lo] = transT[g*32+j_lo, i]
            # built by bouncing transT through HBM, then 4 broadcast loads
            transT_hbm = nc.dram_tensor("transT_hbm", [K, K], fp32)
            nc.sync.dma_start(out=transT_hbm[:], in_=transT)
            trans4 = singles.tile([128, 32, K], fp32)
            for g in range(G):
                src = bass.AP(
                    tensor=transT_hbm, offset=g * 32 * K,
                    ap=[[0, 32], [1, 32 * K]])
                nc.sync.dma_start(
                    out=trans4[g * 32:(g + 1) * 32].rearrange("p a b -> p (a b)"),
                    in_=src)

            # masks (f32)
            lenI = singles.tile([BL, 1], i32)
            nc.sync.dma_start(out=lenI, in_=lengths[:])
            lenF = singles.tile([BL, 1], fp32)
            nc.vector.tensor_copy(out=lenF, in_=lenI)
            mask32 = singles.tile([BL, Tn], fp32)
            nc.vector.tensor_tensor(
                out=mask32, in0=iotaT_f,
                in1=lenF[:].broadcast_to([BL, Tn]), op=ALU.is_lt)
            maskB32 = singles.tile([BL, Tn], fp32)
            nc.gpsimd.tensor_copy(out=maskB32, in_=mask32)
            nc.gpsimd.memset(maskB32[:, 0:1], 0.0)
            maskI = singles.tile([BL, Tn], mybir.dt.uint8)
            nc.vector.tensor_copy(out=maskI, in_=mask32)

            # tags: float; masked variants (masked slot -> -1, never matches iota)
            tagsI = singles.tile([BL, Tn], i32)
            nc.sync.dma_start(out=tagsI, in_=tags[:])
            tagsF = singles.tile([BL, Tn], fp32)
            nc.vector.tensor_copy(out=tagsF, in_=tagsI)
            tagmF = singles.tile([BL, Tn], fp32)  # (tags+1)*mask - 1
            nc.vector.scalar_tensor_tensor(
                out=tagmF, in0=tagsF, scalar=1.0, in1=mask32,
                op0=ALU.add, op1=ALU.mult)
            nc.vector.tensor_scalar_sub(tagmF, tagmF, 1.0)
            tagmPrevF = singles.tile([BL, Tn], fp32)  # masked (by maskB) prev tags
            prevF = singles.tile([BL, Tn], fp32)
            nc.gpsimd.memset(prevF[:, 0:1], 0.0)
            nc.gpsimd.tensor_copy(out=prevF[:, 1:Tn], in_=tagsF[:, 0:Tn - 1])
            nc.vector.scalar_tensor_tensor(
                out=tagmPrevF, in0=prevF, scalar=1.0, in1=maskB32,
                op0=ALU.add, op1=ALU.mult)
            nc.vector.tensor_scalar_sub(tagmPrevF, tagmPrevF, 1.0)

            # tags4m in [(t_hi,b), chunk, t_lo] layout for gold-unary
            tags4 = singles.tile([128, nch, tlo], fp32)
            for th in range(n_thi):
                src_t = bass.AP(
                    tensor=tagmF.tensor, offset=tagmF.offset + th * tlo,
                    ap=[tagmF.ap[0], [chs, nch], [1, tlo]])
                nc.sync.dma_start(out=tags4[th * 32:(th + 1) * 32], in_=src_t)

            # persistent state
            zbuf = singles.tile([BL, Tn], fp32)
            v_state = singles.tile([BL, K], fp32)
            uacc = singles.tile([128, 1], fp32)
            nc.gpsimd.memset(uacc, 0.0)
            pathF = singles.tile([BL, Tn], fp32)

            # ---------------- chunk load + gold-unary ----------------
            def load_chunk(c):
                raw128 = chunkraw_p.tile([128, tlo, K], fp32, tag="raw128")
                for th in range(n_thi):
                    nc.sync.dma_start(
                        out=raw128[th * 32:(th + 1) * 32],
                        in_=logits[:, c * chs + th * tlo: c * chs + (th + 1) * tlo, :])
                raw32 = chunk32_p.tile([BL, chs, K], fp32, tag="raw32")
                nc.sync.dma_start(out=raw32, in_=logits[:, c * chs:(c + 1) * chs, :])
                exp32 = chunk32_p.tile([BL, chs, K], fp32, tag="exp32")
                nc.scalar.activation(out=exp32, in_=raw32, func=AF.Exp)
                return raw128, raw32, exp32

            def gold_unary(c, raw128):
                oh = wide_p.tile([128, tlo, K], fp32, tag="goldoh")
                nc.vector.tensor_tensor(
                    out=oh,
                    in0=iotaK_f[:].unsqueeze(1).broadcast_to([128, tlo, K]),
                    in1=tags4[:, c, :].unsqueeze(2).broadcast_to([128, tlo, K]),
                    op=ALU.is_equal)
                selu = wide_p.tile([128, tlo, K], fp32, tag="selu")
                usr = small_p.tile([128, 1], fp32, tag="usr")
                nc.vector.scalar_tensor_tensor(
                    out=selu, in0=oh, scalar=1.0, in1=raw128,
                    op0=ALU.mult, op1=ALU.mult, accum_out=usr)
                nc.vector.tensor_tensor(out=uacc, in0=uacc, in1=usr, op=ALU.add)

            raw128, raw32, exp32 = load_chunk(0)
            gold_unary(0, raw128)

            # forward init (t=0)
            nc.vector.tensor_reduce(
                out=zbuf[:, 0:1], in_=exp32[:, 0, :], axis=AX.X, op=ALU.add)
            r0 = small_p.tile([BL, 1], fp32, tag="recip")
            nc.vector.reciprocal(out=r0, in_=zbuf[:, 0:1])
            p0 = small_p.tile([BL, K], fp32, tag="pstate")
            nc.scalar.activation(out=p0, in_=exp32[:, 0, :], func=AF.Copy, scale=r0)
            pT_ps = ps.tile([K, BL], fp32, tag="pT")
            nc.tensor.transpose(pT_ps, p0, id32)
            pTs = small_p.tile([K, BL], fp32, tag="pTs")
            nc.scalar.copy(out=pTs, in_=pT_ps)

            # viterbi init
            nc.vector.tensor_copy(out=v_state, in_=raw32[:, 0, :])
            nc.sync.dma_start(out=vitbuf[0], in_=v_state)

            # ---------------- main scan ----------------
            JR = 12  # j_lo split for the M1 reduce: DVE [0:JR), Pool [JR:32)
            for t in range(1, Tn):
                c, tl = divmod(t, chs)
                if tl == 0:
                    raw128, raw32, exp32 = load_chunk(c)
                    gold_unary(c, raw128)

                # ---- forward (lse in exp domain) ----
                qP = ps.tile([BL, K], fp32, tag="qP")
                nc.tensor.matmul(qP, pTs, E, start=True, stop=True)
                u = small_p.tile([BL, K], fp32, tag="ufwd")
                nc.vector.scalar_tensor_tensor(
                    out=u, in0=qP, scalar=1.0, in1=exp32[:, tl, :],
                    op0=ALU.mult, op1=ALU.mult, accum_out=zbuf[:, t:t + 1])
                r = small_p.tile([BL, 1], fp32, tag="recip")
                nc.vector.reciprocal(out=r, in_=zbuf[:, t:t + 1])
                p = small_p.tile([BL, K], fp32, tag="pstate")
                nc.scalar.activation(out=p, in_=u, func=AF.Copy, scale=r)
                pT_ps = ps.tile([K, BL], fp32, tag="pT")
                nc.tensor.transpose(pT_ps, p, id32)
                pTs = small_p.tile([K, BL], fp32, tag="pTs")
                nc.scalar.copy(out=pTs, in_=pT_ps)

                # ---- viterbi step ----
                vr4P = ps.tile([128, K], fp32, tag="vr4P")
                nc.tensor.matmul(vr4P, repid, v_state, start=True, stop=True)
                vr4 = small_p.tile([128, K], fp32, tag="vr4s")
                nc.scalar.copy(out=vr4, in_=vr4P)
                S = wide_p.tile([128, 32, K], fp32, tag="S")
                nc.vector.tensor_tensor(
                    out=S, in0=vr4[:].unsqueeze(1).broadcast_to([128, 32, K]),
                    in1=trans4, op=ALU.add)
                M = small_p.tile([128, 32], fp32, tag="M")
                nc.vector.tensor_reduce(out=M, in_=S, axis=AX.X, op=ALU.max)
                vnP = ps.tile([BL, K], fp32, tag="vnP")
                for g in range(G):
                    nc.tensor.matmul(
                        vnP[:, g * 32:(g + 1) * 32],
                        id128[:, g * 32:(g + 1) * 32], M,
                        start=True, stop=True)
                u2 = small_p.tile([BL, K], fp32, tag="u2")
                nc.vector.tensor_tensor(
                    out=u2, in0=vnP, in1=raw32[:, tl, :], op=ALU.add)
                nc.vector.copy_predicated(
                    out=v_state,
                    mask=maskI[:, t:t + 1].broadcast_to([BL, K]),
                    data=u2)
                # stage the store so the outgoing DMA never blocks the
                # in-place v_state recurrence (Pool is otherwise idle)
                sidx = (t - 1) % 4
                if sidx == 0:
                    vstage = chunk32_p.tile([BL, 4, K], fp32, tag="vstage")
                nc.gpsimd.tensor_copy(out=vstage[:, sidx, :], in_=v_state)
                if sidx == 3 or t == Tn - 1:
                    dst = bass.AP(
                        tensor=vitbuf, offset=(t - sidx) * BL * K,
                        ap=[[K, BL], [BL * K, sidx + 1], [1, K]])
                    nc.sync.dma_start(out=dst, in_=vstage[:, 0:sidx + 1, :])

            # ---------------- forward finalize: log_norm ----------------
            logz = singles.tile([BL, Tn], fp32)
            nc.scalar.activation(out=logz, in_=zbuf, func=AF.Ln)
            nc.vector.tensor_tensor(out=logz, in0=logz, in1=mask32, op=ALU.mult)
            lognorm = singles.tile([BL, 1], fp32)
            nc.vector.tensor_reduce(out=lognorm, in_=logz, axis=AX.X, op=ALU.add)

            # ---------------- gold binary: one-hot pair-count matmuls ----------
            tch = min(128, Tn)
            ntch = Tn // tch
            tagsT = singles.tile([tch, ntch, BL], fp32)
            tagsPT = singles.tile([tch, ntch, BL], fp32)
            for cc in range(ntch):
                for (dst, srcb) in ((tagsT, tagsF), (tagsPT, tagmPrevF)):
                    tp = ps.tile([tch, BL], fp32, tag="pT")
                    nc.tensor.transpose(
                        tp, srcb[:, cc * tch:(cc + 1) * tch], id32)
                    nc.scalar.copy(out=dst[:, cc, :], in_=tp)
            bsc = singles.tile([128, BL], fp32)
            for b in range(BL):
                cP = ps.tile([K, K], fp32, tag="vr4P")
                for cc in range(ntch):
                    ohn = bt_p.tile([tch, K], fp32, tag="ohn")
                    nc.vector.tensor_tensor(
                        out=ohn, in0=iotaK_f[0:tch],
                        in1=tagsT[:, cc, b:b + 1].broadcast_to([tch, K]),
                        op=ALU.is_equal)
                    ohp = bt_p.tile([tch, K], fp32, tag="ohp")
                    nc.vector.tensor_tensor(
                        out=ohp, in0=iotaK_f[0:tch],
                        in1=tagsPT[:, cc, b:b + 1].broadcast_to([tch, K]),
                        op=ALU.is_equal)
                    nc.tensor.matmul(cP, ohp, ohn, start=(cc == 0),
                                     stop=(cc == ntch - 1))
                ct = bt_p.tile([K, K], fp32, tag="ct")
                nc.vector.tensor_tensor(out=ct, in0=cP, in1=transS, op=ALU.mult)
                nc.vector.tensor_reduce(
                    out=bsc[:, b:b + 1], in_=ct, axis=AX.X, op=ALU.add)

            # fold gold scores and loss
            useqP = ps.tile([BL, 1], fp32, tag="vnP")
            nc.tensor.matmul(useqP, fold4, uacc, start=True, stop=True)
            browP = ps.tile([1, BL], fp32, tag="qP")
            nc.tensor.matmul(browP, ones128, bsc, start=True, stop=True)
            brow = small_p.tile([1, BL], fp32, tag="brow")
            nc.scalar.copy(out=brow, in_=browP)
            bseqP = ps.tile([BL, 1], fp32, tag="pT")
            nc.tensor.transpose(bseqP, brow, ones32[0:1, 0:1])
            nllb = small_p.tile([BL, 1], fp32, tag="nllb")
            nc.vector.tensor_tensor(out=nllb, in0=lognorm, in1=useqP,
                                    op=ALU.subtract)
            nc.vector.tensor_tensor(out=nllb, in0=nllb, in1=bseqP,
                                    op=ALU.subtract)
            nllS = small_p.tile([BL, 1], fp32, tag="nllS")
            nc.vector.tensor_copy(out=nllS, in_=nllb)
            lossP = ps.tile([1, 1], fp32, tag="qP")
            nc.tensor.matmul(lossP, nllS, ones32, start=True, stop=True)
            lossS = small_p.tile([1, 1], fp32, tag="lossS")
            nc.scalar.copy(out=lossS, in_=lossP)
            nc.sync.dma_start(out=loss_o[:], in_=lossS)

            # ---------------- viterbi finalize + backtrack ----------------
            mx8 = small_p.tile([BL, 8], fp32, tag="mx8")
            nc.vector.max(out=mx8, in_=v_state)
            vs = small_p.tile([BL, 1], fp32, tag="vs")
            nc.vector.tensor_copy(out=vs, in_=mx8[:, 0:1])
            nc.sync.dma_start(out=vscore_o[:], in_=vs)
            idx8 = small_p.tile([BL, 8], u32, tag="idx8")
            nc.vector.max_index(out=idx8, in_max=mx8, in_values=v_state)
            nc.vector.tensor_copy(out=pathF[:, Tn - 1:Tn], in_=idx8[:, 0:1])

            for t in range(Tn - 1, 0, -1):
                vprev = vload_p.tile([BL, K], fp32, tag="vprev")
                nc.sync.dma_start(out=vprev, in_=vitbuf[t - 1])
                ohb = bt_p.tile([BL, K], fp32, tag="ohb")
                nc.vector.tensor_tensor(
                    out=ohb, in0=iotaK_f[0:BL],
                    in1=pathF[:, t:t + 1].broadcast_to([BL, K]),
                    op=ALU.is_equal)
                ohtP = ps.tile([K, BL], fp32, tag="pT")
                nc.tensor.transpose(ohtP, ohb, id32)
                ohs = bt_p.tile([K, BL], fp32, tag="ohs")
                nc.scalar.copy(out=ohs, in_=ohtP)
                gtP = ps.tile([BL, K], fp32, tag="qP")
                nc.tensor.matmul(gtP, ohs, transT, start=True, stop=True)
                w = bt_p.tile([BL, K], fp32, tag="w")
                nc.vector.tensor_tensor(out=w, in0=gtP, in1=vprev, op=ALU.add)
                wm8 = bt_p.tile([BL, 8], fp32, tag="wm8")
                nc.vector.max(out=wm8, in_=w)
                wi8 = bt_p.tile([BL, 8], u32, tag="wi8")
                nc.vector.max_index(out=wi8, in_max=wm8, in_values=w)
                wf = bt_p.tile([BL, 1], fp32, tag="wf")
                nc.vector.tensor_copy(out=wf, in_=wi8[:, 0:1])
                nc.vector.tensor_copy(out=pathF[:, t - 1:t], in_=pathF[:, t:t + 1])
                nc.vector.copy_predicated(
                    out=pathF[:, t - 1:t], mask=maskI[:, t:t + 1], data=wf)

            pathI = singles.tile([BL, Tn], i32)
            nc.vector.tensor_copy(out=pathI, in_=pathF)
            nc.sync.dma_start(out=path_o[:], in_=pathI)

    nc.finalize()
    return nc


def kernel(logits, tags, lengths, trans):
    from concourse.bass_utils import run_bass_kernel_spmd

    logits = np.ascontiguousarray(logits, dtype=np.float32)
    tags = np.ascontiguousarray(tags, dtype=np.int32)
    lengths = np.ascontiguousarray(lengths, dtype=np.int32)
    trans = np.ascontiguousarray(trans, dtype=np.float32)

    if "nc" not in _BUILD_CACHE:
        _BUILD_CACHE["nc"] = build_bass(T)
    nc = _BUILD_CACHE["nc"]

    in_maps = []
    for c in range(NCORES):
        sl = slice(c * BL, (c + 1) * BL)
        in_maps.append({
            "logits": logits[sl],
            "tags": tags[sl],
            "lengths": lengths[sl].reshape(BL, 1),
            "trans": trans,
        })
    res = run_bass_kernel_spmd(
        nc, in_maps, core_ids=list(range(NCORES)),
        trace=bool(int(os.environ.get("CRF_TRACE", "0"))))
    outs = res.results

    loss = np.float32(sum(float(o["loss_o"][0, 0]) for o in outs))
    path = np.concatenate([o["path_o"] for o in outs], axis=0)
    vscore = np.concatenate([o["vscore_o"][:, 0] for o in outs], axis=0)
    kernel.last_exec_time_ns = res.exec_time_ns
    return loss, path, vscore


kernel.last_exec_time_ns = None
